# revision 1
# baseline (speedup 1.0000x reference)
"""Trainium2 Bass kernel for nn_DagLinkExtractor (sparse_attention).

Math (per batch b, per row i):
  Q = F @ Wq + bq ; K = F @ Wk + bk                     (f16 matmuls)
  exp scale=1/16 folds the 1/sqrt(HD) score scaling into the ACT op
  s_h[i,j] = Q_h[i] . K_h[j]                            (= scores/sqrt(HD))
  gates: u_h = exp(gl_h - max_h gl), gden = sum_h u_h
  K cols of invalid j are zeroed -> s[:, j] = 0, exp adds exactly 1 per
  invalid j>i; a host-computed suffix count c[i] subtracts those from the
  accumulated S_h.  Triangular additive mask (-1e9) on the diagonal block
  kills j<=i before exp.
  w_h = u_h / (gden * (S_h + 1e-30))
  out[i,j] = ln(sum_h p_h[i,j] * w_h + 1e-38), then min-mask with the
  valid-column vector -> exact -1e9 on invalid columns.

Sharding: data-parallel over B, one batch per NeuronCore (8 cores), no
collectives.  Host prep: transpose F, scale Wq, cast matmul operands to
f16, build mask vectors.  Host post: fill the whole lower triangle
(j <= i) with -1e9 (never computed on device).
"""
import numpy as np

import concourse.bass as bass
import concourse.mybir as mybir
import concourse.tile as tile
from concourse import bacc
from concourse.bass_utils import run_bass_kernel_spmd

f32 = mybir.dt.float32
f16 = mybir.dt.float16
F16_NP = np.float16

B, N, HID, NH = 8, 1024, 1024, 4
HD = HID // NH          # 256
NC = HID // 128         # 8 chunks of the hidden/contraction dim
NI = N // 128           # 8 row chunks
NEG = -1000000000.0


def _pin_act_tables():
    """Make natural_log_exp_and_others the only set offering Exp/Ln/Identity
    so bacc emits a single ACT table load instead of ping-ponging between the
    exp-only and ln-only sets every row chunk (~2.7us per switch)."""
    from concourse.hw_specs import get_activation_tables
    aft = mybir.ActivationFunctionType
    tables = get_activation_tables("gen3")  # functools.cache -> shared dict
    keep = "natural_log_exp_and_others"
    if keep in tables:
        for name, funcs in tables.items():
            if name != keep:
                funcs.discard(aft.Exp)
                funcs.discard(aft.Ln)
                funcs.discard(aft.Identity)


def build_nc(variant="full", reps=1):
    _pin_act_tables()
    nc = bacc.Bacc("TRN2", target_bir_lowering=False, debug=False)

    ft_d = nc.dram_tensor("ft", [HID, N], f16, kind="ExternalInput").ap()
    wq_d = nc.dram_tensor("wq", [HID, HID], f16, kind="ExternalInput").ap()
    wk_d = nc.dram_tensor("wk", [HID, HID], f16, kind="ExternalInput").ap()
    wg_d = nc.dram_tensor("wg", [HID, NH], f16, kind="ExternalInput").ap()
    # packed f32 consts: [tri | bq | bk | c]  (128, 128 + 2*NC + NI)
    cp_d = nc.dram_tensor("cp", [128, 128 + 2 * NC + NI], f32,
                          kind="ExternalInput").ap()
    # packed f16 row consts: [ones | bg]  (1, 128 + NH)
    rp_d = nc.dram_tensor("rp", [1, 128 + NH], f16, kind="ExternalInput").ap()
    # mk: 1.0 where valid, 0.0 where !valid (f16) — zeroes K columns
    mk_d = nc.dram_tensor("mk", [1, N], f16, kind="ExternalInput").ap()
    # mv: BIG where valid, -1e9 where !valid — final min-mask (broadcast)
    mv_d = nc.dram_tensor("mv", [1, N], f32, kind="ExternalInput").ap()
    out_d = nc.dram_tensor("out", [N, N], f32, kind="ExternalOutput").ap()

    with tile.TileContext(nc) as tc:
        with tc.tile_pool(name="keep", bufs=1) as keep:

            # ---- persistent SBUF tensors ----
            qt = keep.tile([128, NC, N], f16, tag="qt")   # Q^T (d, i), d-chunked
            kt = keep.tile([128, NC, N], f16, tag="kt")   # K^T (d, j)
            mvb = keep.tile([128, N], f32, tag="mvb")      # broadcast min-mask
            mkb = keep.tile([128, N], f16, tag="mkb")     # broadcast K-col mask
            cp_t = keep.tile([128, 128 + 2 * NC + NI], f32, tag="cp")
            rp_t = keep.tile([1, 128 + NH], f16, tag="rp")
            trir_t = cp_t[:, 0:128]
            bq_t = cp_t[:, 128:128 + NC]
            bk_t = cp_t[:, 128 + NC:128 + 2 * NC]
            c_t = cp_t[:, 128 + 2 * NC:128 + 2 * NC + NI]
            ones_t = rp_t[:, 0:128]
            bg_t = rp_t[:, 128:128 + NH]
            u_t = keep.tile([128, NI, NH], f32, tag="u")   # gate numerators
            gd_t = keep.tile([128, NI], f32, tag="gd")     # gate denominators
            eps_t = keep.tile([128, 1], f32, tag="eps")
            nc.vector.memset(eps_t[:, :], 1e-38)

            def load_consts():
                nc.gpsimd.dma_start(out=cp_t[:, :], in_=cp_d)
                nc.gpsimd.dma_start(out=rp_t[:, :], in_=rp_d)
                nc.gpsimd.dma_start(out=mkb[:, :], in_=bass.AP(
                    tensor=mk_d.tensor, offset=mk_d.offset, ap=[[0, 128], [1, N]]))
                nc.gpsimd.dma_start(out=mvb[:, :], in_=bass.AP(
                    tensor=mv_d.tensor, offset=mv_d.offset, ap=[[0, 128], [1, N]]))

            for _rep in range(reps):
                _emit_body(nc, tc, keep, variant,
                           ft_d, wq_d, wk_d, wg_d, out_d,
                           qt, kt, mvb, mkb, trir_t, ones_t,
                           bg_t, bq_t, bk_t, c_t, u_t, gd_t, eps_t,
                           load_consts if _rep == 0 else None)

    nc.compile()
    return nc


def _emit_body(nc, tc, keep, variant, ft_d, wq_d, wk_d, wg_d, out_d,
               qt, kt, mvb, mkb, trir_t, ones_t, bg_t, bq_t, bk_t, c_t,
               u_t, gd_t, eps_t, load_consts=None):
    with tc.tile_pool(name="wts", bufs=1) as wts, \
         tc.tile_pool(name="psum", bufs=8, space="PSUM") as psp:
        ft = wts.tile([128, NC, N], f16, tag="ft", name="ft")
        wq = wts.tile([128, NC, HID], f16, tag="wq", name="wq")
        wk = wts.tile([128, NC, HID], f16, tag="wk", name="wk")
        wg = wts.tile([128, NC, NH], f16, tag="wg", name="wg")
        # chunked loads, Q-critical tensors first, so projections can start
        # as soon as the first contraction chunks land
        ft_r = ft_d.rearrange("(a p) n -> p a n", p=128)
        wq_r = wq_d.rearrange("(a p) n -> p a n", p=128)
        wk_r = wk_d.rearrange("(a p) n -> p a n", p=128)
        for c in range(NC):
            nc.sync.dma_start(out=ft[:, c, :], in_=ft_r[:, c, :])
            nc.gpsimd.dma_start(out=wq[:, c, :], in_=wq_r[:, c, :])
        if load_consts is not None:
            load_consts()
        for c in range(NC):
            nc.sync.dma_start(out=wk[:, c, :], in_=wk_r[:, c, :])
        nc.gpsimd.dma_start(
            out=wg[:, :, :], in_=wg_d.rearrange("(a p) n -> p a n", p=128))

        # ---- projections: qt[d, i] = sum_c W[c, d] * ft[c, i] (+bias) ----
        # c-outer over all 8 d-chunks at once (one psum bank each) so the PE
        # streams useful matmuls as soon as each input chunk's DMA lands.
        for (w_t, b_t, o_t, msk) in ((wq, bq_t, qt, False),
                                     (wk, bk_t, kt, True)):
            for ih in range(2):
                pss = []
                for dc in range(NC):
                    ps = psp.tile([128, 512], f32, tag="proj", name="proj_ps")
                    pss.append(ps)
                for c in range(NC):
                    for dc in range(NC):
                        nc.tensor.matmul(
                            pss[dc][:, :],
                            w_t[:, c, dc * 128:(dc + 1) * 128],
                            ft[:, c, ih * 512:(ih + 1) * 512],
                            start=(c == 0), stop=(c == NC - 1))
                for dc in range(NC):
                    dst = o_t[:, dc, ih * 512:(ih + 1) * 512]
                    nc.scalar.activation(
                        dst, pss[dc][:, :],
                        mybir.ActivationFunctionType.Identity,
                        bias=b_t[:, dc:dc + 1], scale=1.0)
                    if msk:
                        # zero K columns of invalid j (Pool engine: DVE and
                        # ACT are the scores-phase bottleneck, Pool is idle)
                        nc.vector.tensor_tensor(
                            out=dst, in0=dst,
                            in1=mkb[:, ih * 512:(ih + 1) * 512],
                            op=mybir.AluOpType.mult)

        # ---- gates ----
        for ic in range(NI):
            gps = psp.tile([128, 512], f32, tag="proj", name="gate_ps")[:, 0:NH]
            for c in range(NC):
                nc.tensor.matmul(
                    gps[:, :], ft[:, c, ic * 128:(ic + 1) * 128],
                    wg[:, c, :], start=(c == 0), stop=False)
            nc.tensor.matmul(gps[:, :], ones_t[:, :], bg_t[:, :],
                             start=False, stop=True)
            gnm = keep.tile([128, 1], f32, tag="gnm", name="gnm", bufs=4)
            nc.vector.reduce_max(gnm[:, :], gps[:, :],
                                 axis=mybir.AxisListType.X, negate=True)
            nc.scalar.activation(
                u_t[:, ic, :], gps[:, :],
                mybir.ActivationFunctionType.Exp,
                bias=gnm[:, 0:1], scale=1.0,
                accum_out=gd_t[:, ic:ic + 1])

    if variant == "noscores":
        for ic in range(NI):
            nc.sync.dma_start(out=out_d[ic * 128:(ic + 1) * 128, 0:256],
                              in_=qt[:, ic, 0:512].bitcast(f32))
        return

    # ---- per-row-chunk scores + masked softmax + head mixture ----
    with tc.tile_pool(name="wrk", bufs=3) as wrk, \
         tc.tile_pool(name="spsum", bufs=8, space="PSUM") as sps:
        for ic in (0, 4, 1, 5, 2, 6, 3, 7):
            jt0 = ic // 4            # first live 512-tile of j
            j0 = ic * 128            # first live column
            W = N - j0               # live width
            # per-(jt, head) chains: MMs -> tri -> exp.  jt-outer keeps
            # at most 4 psum banks per group so chunks pipeline across ic.
            p_ts = [wrk.tile([128, W], f32, tag=f"p{h}", name=f"p{h}")
                    for h in range(NH)]
            sa = wrk.tile([128, 2, NH], f32, tag="sa", name="sa")
            for k, jt in enumerate(range(jt0, 2)):
                lo = j0 - jt * 512 if jt == jt0 else 0
                dst0 = jt * 512 + lo - j0
                for h in range(NH):
                    ps = sps.tile([128, 512], f32, tag="ss", name="ss")
                    for t in range(2):
                        dc = 2 * h + t
                        nc.tensor.matmul(
                            ps[:, lo:512],
                            qt[:, dc, ic * 128:(ic + 1) * 128],
                            kt[:, dc, jt * 512 + lo:(jt + 1) * 512],
                            start=(t == 0), stop=(t == 1))
                    # triangular additive mask on the diagonal 128 cols
                    if jt == jt0:
                        # stays on DVE: Pool cannot access PSUM
                        nc.vector.tensor_tensor(
                            out=ps[:, lo:lo + 128], in0=ps[:, lo:lo + 128],
                            in1=trir_t, op=mybir.AluOpType.add)
                    # scale folds the 1/sqrt(HD)=1/16 score scaling into exp
                    nc.scalar.activation(
                        p_ts[h][:, dst0:(jt + 1) * 512 - j0],
                        ps[:, lo:512],
                        mybir.ActivationFunctionType.Exp,
                        bias=0.0, scale=0.0625,
                        accum_out=sa[:, k, h:h + 1])

            if variant == "nomix":
                nc.sync.dma_start(
                    out=out_d[ic * 128:(ic + 1) * 128, j0:j0 + W // 2],
                    in_=p_ts[0][:, :].bitcast(f32))
                continue

            # s4 = (sa0 - c) + sa1: true S_h after removing exp(0)=1 terms
            # from zeroed invalid K columns (c[i] = #{j>i : !valid[j]})
            s4 = wrk.tile([128, NH], f32, tag="s4", name="s4")
            if jt0 == 0:
                nc.vector.scalar_tensor_tensor(
                    out=s4[:, :], in0=sa[:, 0, :],
                    scalar=c_t[:, ic:ic + 1], in1=sa[:, 1, :],
                    op0=mybir.AluOpType.subtract, op1=mybir.AluOpType.add)
            else:
                nc.vector.tensor_scalar(
                    out=s4[:, :], in0=sa[:, 0, :],
                    scalar1=c_t[:, ic:ic + 1], scalar2=None,
                    op0=mybir.AluOpType.subtract)
            # w_h = u_h / (gden * (S_h + 1e-30))
            m4 = wrk.tile([128, NH], f32, tag="m4", name="m4")
            nc.vector.tensor_scalar(
                out=m4[:, :], in0=s4[:, :], scalar1=1e-30,
                scalar2=gd_t[:, ic:ic + 1],
                op0=mybir.AluOpType.add, op1=mybir.AluOpType.mult)
            r4 = wrk.tile([128, NH], f32, tag="r4", name="r4")
            nc.vector.reciprocal(out=r4[:, :], in_=m4[:, :])
            w4 = wrk.tile([128, NH], f32, tag="w4", name="w4")
            nc.vector.tensor_tensor(out=w4[:, :], in0=u_t[:, ic, :],
                                    in1=r4[:, :], op=mybir.AluOpType.mult)

            # mixture: acc = sum_h p_h * w_h (f32: p/S must cancel to ~1ulp
            # for rows whose softmax is a single element, where expected=0).
            # The [0, 6e4] clamp keeps ln() finite (dead-row w_h ~1e30 and
            # exp(0)-table negatives) so the min-mask always lands.
            acc = wrk.tile([128, W], f32, tag="acc", name="acc")
            nc.vector.tensor_scalar(
                out=acc[:, :], in0=p_ts[0][:, :], scalar1=w4[:, 0:1],
                scalar2=None, op0=mybir.AluOpType.mult)
            for h in range(1, NH):
                nc.vector.scalar_tensor_tensor(
                    out=acc[:, :], in0=p_ts[h][:, :], scalar=w4[:, h:h + 1],
                    in1=acc[:, :], op0=mybir.AluOpType.mult,
                    op1=mybir.AluOpType.add)
            nc.vector.tensor_scalar(
                out=acc[:, :], in0=acc[:, :], scalar1=0.0, scalar2=60000.0,
                op0=mybir.AluOpType.max, op1=mybir.AluOpType.min)

            # out = ln(acc + 1e-38), then min-mask -> exact -1e9 on !valid
            o_t = wrk.tile([128, W], f32, tag="o", name="o")
            nc.scalar.activation(o_t[:, :], acc[:, :],
                                 mybir.ActivationFunctionType.Ln,
                                 bias=eps_t[:, 0:1], scale=1.0)
            nc.vector.tensor_tensor(out=o_t[:, :], in0=o_t[:, :],
                                    in1=mvb[:, j0:], op=mybir.AluOpType.min)
            nc.sync.dma_start(out=out_d[ic * 128:(ic + 1) * 128, j0:],
                              in_=o_t[:, :])


_NC_CACHE = None


def _get_nc():
    global _NC_CACHE
    if _NC_CACHE is None:
        _NC_CACHE = build_nc()
    return _NC_CACHE


def make_in_maps(features, valid_mask, Wq, bq, Wk, bk, Wg, bg):
    features = np.asarray(features, dtype=np.float32)
    valid_mask = np.asarray(valid_mask).astype(bool)
    wq_b = np.asarray(Wq, np.float32).astype(F16_NP)
    wk_b = np.asarray(Wk, np.float32).astype(F16_NP)
    wg_b = np.asarray(Wg, np.float32).astype(F16_NP)
    bq_s = np.asarray(bq, np.float32).reshape(NC, 128).T.copy()
    bk_s = np.asarray(bk, np.float32).reshape(NC, 128).T.copy()
    bg_b = np.asarray(bg, np.float32).reshape(1, NH).astype(F16_NP)
    ones = np.ones((1, 128), F16_NP)
    c = np.arange(128)[None, :]
    rr = np.arange(128)[:, None]
    tri = np.where(c > rr, 0.0, NEG).astype(np.float32)
    rp = np.concatenate([ones, bg_b], axis=1)
    in_maps = []
    for b_i in range(B):
        vm = valid_mask[b_i]
        mk = vm.astype(np.float32).astype(F16_NP).reshape(1, N)
        mv = np.where(vm, 3.0e9, NEG).astype(np.float32).reshape(1, N)
        # c[i] = #invalid j > i  (suffix count of ~valid)
        inv = (~vm).astype(np.int64)
        suf = np.concatenate([np.cumsum(inv[::-1])[::-1][1:], [0]])
        c_m = suf.astype(np.float32).reshape(NI, 128).T.copy()
        cp = np.concatenate([tri, bq_s, bk_s, c_m], axis=1)
        in_maps.append({
            "ft": features[b_i].T.astype(F16_NP),
            "wq": wq_b, "wk": wk_b, "wg": wg_b,
            "cp": cp, "rp": rp, "mk": mk, "mv": mv,
        })
    return in_maps


_TRI_DEAD = None


def gather_out(results):
    global _TRI_DEAD
    out = np.empty((B, N, N), dtype=np.float32)
    for b_i in range(B):
        out[b_i] = results[b_i]["out"]
    # the whole lower triangle (j <= i) is never valid output: the device
    # only computes j > i entries (block rows start at j0 = 128*floor(i/128))
    if _TRI_DEAD is None:
        _TRI_DEAD = np.arange(N)[None, :] <= np.arange(N)[:, None]
    out[:, _TRI_DEAD] = np.float32(NEG)
    return out


def kernel(features, valid_mask, Wq, bq, Wk, bk, Wg, bg):
    nc = _get_nc()
    in_maps = make_in_maps(features, valid_mask, Wq, bq, Wk, bk, Wg, bg)
    res = run_bass_kernel_spmd(nc, in_maps, core_ids=list(range(B)))
    return gather_out(res.results)



# revision 25
# speedup vs baseline: 5.2448x; 5.2448x over previous
"""Trainium2 Bass kernel for nn_DagLinkExtractor (sparse_attention).

Math (per batch b, per row i):
  Q = F @ Wq + bq ; K = F @ Wk + bk                     (f16 matmuls)
  exp scale=1/16 folds the 1/sqrt(HD) score scaling into the ACT op
  s_h[i,j] = Q_h[i] . K_h[j]                            (= scores/sqrt(HD))
  gates: u_h = exp(gl_h - max_h gl), gden = sum_h u_h
  K cols of invalid j are zeroed -> s[:, j] = 0, exp adds exactly 1 per
  invalid j>i; a host-computed suffix count c[i] subtracts those from the
  accumulated S_h.  Triangular additive mask (-1e9) on the diagonal block
  kills j<=i before exp.
  w_h = u_h / (gden * (S_h + 1e-30))
  out[i,j] = ln(sum_h p_h[i,j] * w_h + 1e-38)

Sharding: data-parallel over B, one batch per NeuronCore (8 cores), no
collectives.

I/O signature is ONE input + ONE output per core: the per-exec PJRT/axon
dispatch cost scales with the argument count (~0.1ms per operand), which
dominated the old 8-input layout.  All inputs are packed into a single
f16 DRAM blob (f32 consts ride as bitcast f16 byte pairs); the output is
a packed f16 [128, 4608] tile holding only the computed upper-block rows.
Host post: unpack, cast to f32, fill lower triangle / invalid columns /
dead rows with exactly -1e9 (never computed on device).
"""
import numpy as np

import concourse.bass as bass
import concourse.mybir as mybir
import concourse.tile as tile
from concourse import bacc
from concourse.bass_utils import run_bass_kernel_spmd

f32 = mybir.dt.float32
f16 = mybir.dt.float16
F16_NP = np.float16

B, N, HID, NH = 8, 1024, 1024, 4
HD = HID // NH          # 256
NC = HID // 128         # 8 chunks of the hidden/contraction dim
NI = N // 128           # 8 row chunks
NEG = -1000000000.0

# --- packed input blob layout: f16 [1024, BLOB_C] ---
FT0 = 0               # features^T  [1024, 1024]
WQ0 = 1024            # Wq          [1024, 1024]
WK0 = 2048            # Wk          [1024, 1024]
WG0 = 3072            # Wg          [1024, 4]
CP0 = 3076            # f32 consts as f16 byte pairs: rows 0:128, 304 cols
CPW = 2 * (128 + 2 * NC + NI)   # tri(128) | bq(8) | bk(8) | c(8) f32 -> 304
RP0 = CP0 + CPW       # row consts f16: row 0: ones(128) | bg(4)
RPW = 128 + NH
MK0 = RP0 + RPW       # K-col valid mask f16 (1.0/0.0): row 0, N cols
BLOB_C = MK0 + N      # 4536

# --- packed output layout: f16 [128, OUT_C] ---
# row-chunk ic contributes [128, N - 128*ic] at column OUT_OFF[ic]
OUT_W = [N - 128 * ic for ic in range(NI)]
OUT_OFF = [sum(OUT_W[:ic]) for ic in range(NI)]
OUT_C = sum(OUT_W)    # 4608


def _pin_act_tables():
    """Make natural_log_exp_and_others the only set offering Exp/Ln/Identity
    so bacc emits a single ACT table load instead of ping-ponging between the
    exp-only and ln-only sets every row chunk (~2.7us per switch)."""
    from concourse.hw_specs import get_activation_tables
    aft = mybir.ActivationFunctionType
    tables = get_activation_tables("gen3")  # functools.cache -> shared dict
    keep = "natural_log_exp_and_others"
    if keep in tables:
        for name, funcs in tables.items():
            if name != keep:
                funcs.discard(aft.Exp)
                funcs.discard(aft.Ln)
                funcs.discard(aft.Identity)


def _dchunk(d, row0, col0, nrows, ncols, row_stride=BLOB_C):
    """2-D [nrows, ncols] view into the packed DRAM blob."""
    return bass.AP(tensor=d.tensor, offset=d.offset + row0 * row_stride + col0,
                   ap=[[row_stride, nrows], [1, ncols]])


def build_nc(variant="full", reps=1):
    _pin_act_tables()
    nc = bacc.Bacc("TRN2", target_bir_lowering=False, debug=False)

    in_d = nc.dram_tensor("inp", [1024, BLOB_C], f16, kind="ExternalInput").ap()
    out_d = nc.dram_tensor("out", [128, OUT_C], f16, kind="ExternalOutput").ap()

    with tile.TileContext(nc) as tc:
        with tc.tile_pool(name="keep", bufs=1) as keep:

            # ---- persistent SBUF tensors ----
            qt = keep.tile([128, NC, N], f16, tag="qt")   # Q^T (d, i), d-chunked
            kt = keep.tile([128, NC, N], f16, tag="kt")   # K^T (d, j)
            mkb = keep.tile([128, N], f16, tag="mkb")     # broadcast K-col mask
            cp16 = keep.tile([128, CPW], f16, tag="cp")
            cp_t = cp16[:, :].bitcast(f32)                # [128, 152] f32 view
            rp_t = keep.tile([1, RPW], f16, tag="rp")
            trir_t = cp_t[:, 0:128]
            bq_t = cp_t[:, 128:128 + NC]
            bk_t = cp_t[:, 128 + NC:128 + 2 * NC]
            c_t = cp_t[:, 128 + 2 * NC:128 + 2 * NC + NI]
            ones_t = rp_t[:, 0:128]
            bg_t = rp_t[:, 128:128 + NH]
            u_t = keep.tile([128, NI, NH], f32, tag="u")   # gate numerators
            gd_t = keep.tile([128, NI], f32, tag="gd")     # gate denominators
            eps_t = keep.tile([128, 1], f32, tag="eps")
            nc.vector.memset(eps_t[:, :], 1e-38)
            # dummy ACT op: pulls the 1.3us activation-table load (lazy,
            # on first ACT use) into the DMA ramp where ACT is idle,
            # instead of stalling the first PSUM drains mid-projection
            warm_t = keep.tile([128, 1], f32, tag="warm")
            nc.scalar.activation(warm_t[:, :], eps_t[:, :],
                                 mybir.ActivationFunctionType.Identity,
                                 bias=0.0, scale=1.0)

            def load_consts():
                nc.gpsimd.dma_start(out=cp16[:, :],
                                    in_=_dchunk(in_d, 0, CP0, 128, CPW))
                nc.gpsimd.dma_start(out=rp_t[:, :],
                                    in_=_dchunk(in_d, 0, RP0, 1, RPW))
                nc.gpsimd.dma_start(out=mkb[:, :], in_=bass.AP(
                    tensor=in_d.tensor, offset=in_d.offset + MK0,
                    ap=[[0, 128], [1, N]]))

            for _rep in range(reps):
                _emit_body(nc, tc, keep, variant, in_d, out_d,
                           qt, kt, mkb, trir_t, ones_t,
                           bg_t, bq_t, bk_t, c_t, u_t, gd_t, eps_t,
                           load_consts if _rep == 0 else None)

    nc.compile()
    return nc


def _scores_ic(nc, wrk, sps, variant, out_d,
               qt, kt, trir_t, c_t, u_t, gd_t, eps_t, ic, wide, tagp="",
               pstag="ss"):
    """Scores + masked softmax + head mixture for one 128-row chunk.

    wide=True (ic < 4): one 2-bank [128, 1024] psum per head, both
    j-halves matmul'd into it, ONE exp op spanning banks (halves the ACT
    instruction + accum-read count).  wide=False (ic >= 4): single live
    j-half, 1-bank psum.
    """
    j0 = ic * 128            # first live column
    W = N - j0               # live width
    p_ts = [wrk.tile([128, W], f32, tag=f"{tagp}p{h}", name=f"p{h}")
            for h in range(NH)]
    sa = wrk.tile([128, NH], f32, tag=tagp + "sa", name="sa")
    for h in range(NH):
        if wide:
            ps = sps.tile([128, 1024], f32, tag=pstag, name="ss")
            for jt in range(2):
                lo = j0 if jt == 0 else 512
                for t in range(2):
                    dc = 2 * h + t
                    nc.tensor.matmul(
                        ps[:, lo:(jt + 1) * 512],
                        qt[:, dc, ic * 128:(ic + 1) * 128],
                        kt[:, dc, lo:(jt + 1) * 512],
                        start=(t == 0), stop=(t == 1))
            src_ap = ps[:, j0:1024]
        else:
            ps = sps.tile([128, 512], f32, tag=pstag, name="ss")
            lo = j0 - 512
            for t in range(2):
                dc = 2 * h + t
                nc.tensor.matmul(
                    ps[:, lo:512],
                    qt[:, dc, ic * 128:(ic + 1) * 128],
                    kt[:, dc, j0:1024],
                    start=(t == 0), stop=(t == 1))
            src_ap = ps[:, lo:512]
        # triangular additive mask on the diagonal 128 cols
        # (stays on DVE: Pool cannot access PSUM)
        loc = j0 if wide else j0 - 512
        nc.vector.tensor_tensor(
            out=ps[:, loc:loc + 128], in0=ps[:, loc:loc + 128],
            in1=trir_t, op=mybir.AluOpType.add)
        # scale folds the 1/sqrt(HD)=1/16 score scaling into exp
        nc.scalar.activation(
            p_ts[h][:, :], src_ap,
            mybir.ActivationFunctionType.Exp,
            bias=0.0, scale=0.0625,
            accum_out=sa[:, h:h + 1])

    if variant == "nomix":
        nc.sync.dma_start(
            out=out_d[:, 0:W], in_=p_ts[0][:, 0:W // 2].bitcast(f16))
        return

    # s4 = sa - c: true S_h after removing exp(0)=1 terms from zeroed
    # invalid K columns (c[i] = #{j>i : !valid[j]})
    s4 = wrk.tile([128, NH], f32, tag=tagp + "s4", name="s4")
    nc.vector.tensor_scalar(
        out=s4[:, :], in0=sa[:, :],
        scalar1=c_t[:, ic:ic + 1], scalar2=None,
        op0=mybir.AluOpType.subtract)
    # w_h = u_h / (gden * (S_h + 1e-30))
    m4 = wrk.tile([128, NH], f32, tag=tagp + "m4", name="m4")
    nc.vector.tensor_scalar(
        out=m4[:, :], in0=s4[:, :], scalar1=1e-30,
        scalar2=gd_t[:, ic:ic + 1],
        op0=mybir.AluOpType.add, op1=mybir.AluOpType.mult)
    r4 = wrk.tile([128, NH], f32, tag=tagp + "r4", name="r4")
    nc.vector.reciprocal(out=r4[:, :], in_=m4[:, :])
    w4 = wrk.tile([128, NH], f32, tag=tagp + "w4", name="w4")
    nc.vector.tensor_tensor(out=w4[:, :], in0=u_t[:, ic, :],
                            in1=r4[:, :], op=mybir.AluOpType.mult)

    # mixture: acc = sum_h p_h * w_h (f32: p/S must cancel to ~1ulp
    # for rows whose softmax is a single element, where expected=0).
    # The [0, 6e4] clamp keeps ln() finite (dead-row w_h ~1e30 and
    # exp(0)-table negatives).
    acc = wrk.tile([128, W], f32, tag=tagp + "acc", name="acc")
    nc.vector.tensor_scalar(
        out=acc[:, :], in0=p_ts[0][:, :], scalar1=w4[:, 0:1],
        scalar2=None, op0=mybir.AluOpType.mult)
    for h in range(1, NH):
        nc.vector.scalar_tensor_tensor(
            out=acc[:, :], in0=p_ts[h][:, :], scalar=w4[:, h:h + 1],
            in1=acc[:, :], op0=mybir.AluOpType.mult,
            op1=mybir.AluOpType.add)
    nc.vector.tensor_scalar(
        out=acc[:, :], in0=acc[:, :], scalar1=0.0, scalar2=60000.0,
        op0=mybir.AluOpType.max, op1=mybir.AluOpType.min)

    # out = ln(acc + 1e-38) -> f16 packed output row-chunk
    o16 = wrk.tile([128, W], f16, tag=tagp + "o", name="o")
    nc.scalar.activation(o16[:, :], acc[:, :],
                         mybir.ActivationFunctionType.Ln,
                         bias=eps_t[:, 0:1], scale=1.0)
    nc.sync.dma_start(out=out_d[:, OUT_OFF[ic]:OUT_OFF[ic] + W],
                      in_=o16[:, :])


def _emit_body(nc, tc, keep, variant, in_d, out_d,
               qt, kt, mkb, trir_t, ones_t, bg_t, bq_t, bk_t, c_t,
               u_t, gd_t, eps_t, load_consts=None):
    # Phase plan (per-engine queues execute in emission order):
    #   PE : K-proj(ih0,ih1) -> Q-proj(ih0) -> gates -> scores ic 0..3
    #        -> Q-proj(ih1) -> scores ic 4..7
    # so the heavy ACT/DVE tail of the wide chunks (exp + mixture + ln)
    # overlaps the high-half Q projection the PE still has queued, and
    # the post-matmul drain tail belongs to the cheap narrow chunks.
    # PSUM budget scope A: proj ring 4 banks + 2x 2-bank wide tiles = 8;
    # scope B (after A closes): 4x 1-bank narrow tiles.
    with tc.tile_pool(name="wts", bufs=1) as wts, \
         tc.tile_pool(name="psum", bufs=4, space="PSUM") as psp, \
         tc.tile_pool(name="spsumw", bufs=2, space="PSUM") as spsw, \
         tc.tile_pool(name="wrk", bufs=3) as wrk:
        ft = wts.tile([128, NC, N], f16, tag="ft", name="ft")
        wq = wts.tile([128, NC, HID], f16, tag="wq", name="wq")
        wk = wts.tile([128, NC, HID], f16, tag="wk", name="wk")
        wg = wts.tile([128, NC, NH], f16, tag="wg", name="wg")
        # chunked loads, K-critical tensors first, so the K projection
        # can start as soon as the first contraction chunks land
        for c in range(NC):
            nc.sync.dma_start(out=ft[:, c, :],
                              in_=_dchunk(in_d, c * 128, FT0, 128, N))
            nc.gpsimd.dma_start(out=wk[:, c, :],
                                in_=_dchunk(in_d, c * 128, WK0, 128, HID))
            if c == 3 and load_consts is not None:
                load_consts()
        for c in range(NC):
            nc.gpsimd.dma_start(out=wq[:, c, :],
                                in_=_dchunk(in_d, c * 128, WQ0, 128, HID))
        # wg: one 3-dim DMA [p, a, n]
        nc.gpsimd.dma_start(out=wg[:, :, :], in_=bass.AP(
            tensor=in_d.tensor, offset=in_d.offset + WG0,
            ap=[[BLOB_C, 128], [128 * BLOB_C, NC], [1, NH]]))

        # ---- projections: qt[d, i] = sum_c W[c, d] * ft[c, i] (+bias) ----
        def drain(o_t, b_t, msk, ih, dc, ps):
            dst = o_t[:, dc, ih * 512:(ih + 1) * 512]
            if msk:
                # fused bias-add + invalid-column zeroing in ONE DVE op
                nc.vector.scalar_tensor_tensor(
                    out=dst, in0=ps[:, :],
                    scalar=b_t[:, dc:dc + 1],
                    in1=mkb[:, ih * 512:(ih + 1) * 512],
                    op0=mybir.AluOpType.add,
                    op1=mybir.AluOpType.mult)
            else:
                nc.scalar.activation(
                    dst, ps[:, :],
                    mybir.ActivationFunctionType.Identity,
                    bias=b_t[:, dc:dc + 1], scale=1.0)

        def proj(w_t, b_t, o_t, msk, ih, c_outer=False, dcs=None):
            # c_outer: first 4 dc chunks c-outermost so the first matmuls
            # need only the first ft/w DMA chunks (DMA-ramp friendly);
            # the rest go c-inner so each bank drains while the next
            # streams.
            if c_outer:
                pss = [psp.tile([128, 512], f32, tag="proj",
                                name="proj_ps") for _ in range(4)]
                for c in range(NC):
                    for gi in range(4):
                        nc.tensor.matmul(
                            pss[gi][:, :],
                            w_t[:, c, gi * 128:(gi + 1) * 128],
                            ft[:, c, ih * 512:(ih + 1) * 512],
                            start=(c == 0), stop=(c == NC - 1))
                for gi in range(4):
                    drain(o_t, b_t, msk, ih, gi, pss[gi])
            for dc in (dcs if dcs is not None
                       else range(4 if c_outer else 0, NC)):
                ps = psp.tile([128, 512], f32, tag="proj", name="proj_ps")
                for c in range(NC):
                    nc.tensor.matmul(
                        ps[:, :],
                        w_t[:, c, dc * 128:(dc + 1) * 128],
                        ft[:, c, ih * 512:(ih + 1) * 512],
                        start=(c == 0), stop=(c == NC - 1))
                drain(o_t, b_t, msk, ih, dc, ps)

        proj(wk, bk_t, kt, True, 0, c_outer=True)
        proj(wk, bk_t, kt, True, 1)
        proj(wq, bq_t, qt, False, 0)

        # ---- gates ----
        for ic in range(NI):
            gps = psp.tile([128, 512], f32, tag="proj", name="gate_ps")[:, 0:NH]
            for c in range(NC):
                nc.tensor.matmul(
                    gps[:, :], ft[:, c, ic * 128:(ic + 1) * 128],
                    wg[:, c, :], start=(c == 0), stop=False)
            nc.tensor.matmul(gps[:, :], ones_t[:, :], bg_t[:, :],
                             start=False, stop=True)
            gnm = keep.tile([128, 1], f32, tag="gnm", name="gnm", bufs=4)
            nc.vector.reduce_max(gnm[:, :], gps[:, :],
                                 axis=mybir.AxisListType.X, negate=True)
            nc.scalar.activation(
                u_t[:, ic, :], gps[:, :],
                mybir.ActivationFunctionType.Exp,
                bias=gnm[:, 0:1], scale=1.0,
                accum_out=gd_t[:, ic:ic + 1])

        if variant != "noscores":
            # ---- wide row-chunks interleaved with Q-ih1 projection
            # chunks: the PE fills its ACT/DVE-paced idle slots with
            # projection matmuls, and the drains spread out instead of
            # gating the narrow phase all at once ----
            for ic in (0, 1, 2, 3):
                _scores_ic(nc, wrk, spsw, variant, out_d,
                           qt, kt, trir_t, c_t, u_t, gd_t, eps_t, ic,
                           wide=True)
                proj(wq, bq_t, qt, False, 1, dcs=(2 * ic, 2 * ic + 1))
        else:
            proj(wq, bq_t, qt, False, 1)

    if variant == "noscores":
        for ic in range(NI):
            nc.sync.dma_start(out=out_d[:, ic * 512:(ic + 1) * 512],
                              in_=qt[:, ic, 0:512])
        return

    # ---- narrow high row-chunks: cheap per-chunk chains, short tail ----
    with tc.tile_pool(name="wrk2", bufs=3) as wrk2, \
         tc.tile_pool(name="spsumn", bufs=4, space="PSUM") as spsn:
        for ic in (4, 5, 6, 7):
            _scores_ic(nc, wrk2, spsn, variant, out_d,
                       qt, kt, trir_t, c_t, u_t, gd_t, eps_t, ic,
                       wide=False, tagp="t")


_NC_CACHE = None


def _get_nc():
    global _NC_CACHE
    if _NC_CACHE is None:
        _NC_CACHE = build_nc()
    return _NC_CACHE


def make_in_maps(features, valid_mask, Wq, bq, Wk, bk, Wg, bg):
    features = np.asarray(features, dtype=np.float32)
    valid_mask = np.asarray(valid_mask).astype(bool)
    wq_b = np.asarray(Wq, np.float32).astype(F16_NP)
    wk_b = np.asarray(Wk, np.float32).astype(F16_NP)
    wg_b = np.asarray(Wg, np.float32).astype(F16_NP)
    bq_s = np.asarray(bq, np.float32).reshape(NC, 128).T.copy()
    bk_s = np.asarray(bk, np.float32).reshape(NC, 128).T.copy()
    bg_b = np.asarray(bg, np.float32).reshape(1, NH).astype(F16_NP)
    ones = np.ones((1, 128), F16_NP)
    col = np.arange(128)[None, :]
    rr = np.arange(128)[:, None]
    tri = np.where(col > rr, 0.0, NEG).astype(np.float32)
    rp = np.concatenate([ones, bg_b], axis=1)
    in_maps = []
    for b_i in range(B):
        vm = valid_mask[b_i]
        mk = vm.astype(np.float32).astype(F16_NP)
        # c[i] = #invalid j > i  (suffix count of ~valid)
        inv = (~vm).astype(np.int64)
        suf = np.concatenate([np.cumsum(inv[::-1])[::-1][1:], [0]])
        c_m = suf.astype(np.float32).reshape(NI, 128).T.copy()
        cp = np.ascontiguousarray(
            np.concatenate([tri, bq_s, bk_s, c_m], axis=1))
        blob = np.zeros((1024, BLOB_C), F16_NP)
        blob[:, FT0:FT0 + N] = features[b_i].T.astype(F16_NP)
        blob[:, WQ0:WQ0 + HID] = wq_b
        blob[:, WK0:WK0 + HID] = wk_b
        blob[:, WG0:WG0 + NH] = wg_b
        blob[0:128, CP0:CP0 + CPW] = cp.view(F16_NP)
        blob[0:1, RP0:RP0 + RPW] = rp
        blob[0, MK0:MK0 + N] = mk
        in_maps.append({"inp": blob})
    return in_maps


_TRI_DEAD = None


def gather_out(results, valid_mask):
    global _TRI_DEAD
    if _TRI_DEAD is None:
        _TRI_DEAD = np.arange(N)[None, :] <= np.arange(N)[:, None]
    valid_mask = np.asarray(valid_mask).astype(bool)
    out = np.empty((B, N, N), dtype=np.float32)
    for b_i in range(B):
        packed = results[b_i]["out"]
        ob = out[b_i]
        for ic in range(NI):
            j0 = 128 * ic
            ob[j0:j0 + 128, j0:] = packed[:, OUT_OFF[ic]:OUT_OFF[ic] + OUT_W[ic]]
        # never computed on device: lower triangle (j <= i), invalid
        # columns, and dead rows (no valid j > i) are exactly -1e9
        ob[_TRI_DEAD] = np.float32(NEG)
        vm = valid_mask[b_i]
        ob[:, ~vm] = np.float32(NEG)
        inv = (~vm).astype(np.int64)
        suf = np.concatenate([np.cumsum(inv[::-1])[::-1][1:], [0]])
        n_after = (N - 1) - np.arange(N)
        dead = suf == n_after
        ob[dead, :] = np.float32(NEG)
    return out


def kernel(features, valid_mask, Wq, bq, Wk, bk, Wg, bg):
    nc = _get_nc()
    in_maps = make_in_maps(features, valid_mask, Wq, bq, Wk, bk, Wg, bg)
    res = run_bass_kernel_spmd(nc, in_maps, core_ids=list(range(B)))
    return gather_out(res.results, valid_mask)
